# revision 1
# baseline (speedup 1.0000x reference)
"""Trainium2 Bass kernel for the octonion causal self-attention block.

Strategy (8 NeuronCores, SPMD):
  Each core owns one octonion component c (= heads 2c, 2c+1).
  - Host prep: ternary-quantize weights exactly as the reference does,
    assemble the effective [C, C] block matrices (sign/permutation
    combine folded in), permute q/k output channels into RoPE
    split-layout (real|imag halves), fold 1/sqrt(HD) into wq, and
    transpose x to channel-major xT.  All heavy FLOPs stay on device.
  - Device phase 1: projections qT/kT (channel-major) + v (natural) for
    the core's component, streaming xT once; RoPE applied on the fly.
  - Device phase 2: causal attention for the core's 4 (batch, head)
    pairs: S^T = K^T Q per s-tile, exp (no max subtraction -- scores are
    O(1) bounded), triangular mask on the diagonal tile, P^T V with an
    appended ones-column in V giving the softmax denominator for free,
    per-partition normalize, PE-transpose back to channel-major yT
    (kept in SBUF).
  - Device phase 3: PARTIAL output projection: out_partial^T =
    Wo[rows of component c]^T @ yT_c -- needs only local data, so no
    collective at all.  The host sums the 8 partial [C, B*T] outputs
    while unsharding (that sum is the tensor-parallel "unshard").
"""

import numpy as np
import ml_dtypes

import concourse.bass as bass
import concourse.tile as tile
from concourse import bacc, mybir
from concourse.bass_utils import run_bass_kernel_spmd
from concourse.masks import make_identity

# ---------------------------------------------------------------- problem dims
B, T_FULL, C, H = 2, 2048, 2048, 16
HD = C // H          # 128
P = C // 8           # 256
N_CORES = 8
KT = C // 128        # 16 contraction k-tiles

OCT_SIGN = np.array([
    [1, 1, 1, 1, 1, 1, 1, 1],
    [1,-1, 1,-1, 1,-1,-1, 1],
    [1,-1,-1, 1, 1, 1,-1,-1],
    [1, 1,-1,-1, 1,-1, 1,-1],
    [1,-1,-1,-1,-1, 1, 1, 1],
    [1, 1,-1, 1,-1,-1,-1, 1],
    [1, 1, 1,-1,-1, 1,-1,-1],
    [1,-1, 1, 1,-1,-1, 1,-1]], dtype=np.float32)
OCT_IDX = np.array([
    [0,1,2,3,4,5,6,7],
    [1,0,3,2,5,4,7,6],
    [2,3,0,1,6,7,4,5],
    [3,2,1,0,7,6,5,4],
    [4,5,6,7,0,1,2,3],
    [5,4,7,6,1,0,3,2],
    [6,7,4,5,2,3,0,1],
    [7,6,5,4,3,2,1,0]], dtype=np.int32)
_COMB = np.zeros((8, 8, 8), np.float32)
for _i in range(8):
    for _j in range(8):
        _COMB[OCT_IDX[_i, _j], _i, _j] = OCT_SIGN[_i, _j]

BF16 = ml_dtypes.bfloat16


# ------------------------------------------------------------------- host prep
def _ternary(W: np.ndarray) -> np.ndarray:
    """Exact replica of the reference's ternary quantization (fwd value)."""
    try:
        import jax
        import jax.numpy as jnp
        cpu = jax.local_devices(backend="cpu")[0]
        with jax.default_device(cpu):
            Wj = jnp.asarray(W)
            s = jnp.mean(jnp.abs(Wj)) + 1e-8
            q = jnp.round(jnp.clip(Wj / s, -1.0, 1.0)) * s
            return np.asarray(q)
    except Exception:
        s = np.float32(np.mean(np.abs(W.astype(np.float32)))) + np.float32(1e-8)
        return (np.rint(np.clip(W / s, -1.0, 1.0)) * s).astype(np.float32)


def _build_w_eff(W: np.ndarray) -> np.ndarray:
    """[8,P,P] component weights -> effective [C, C] with octonion combine."""
    Wq = _ternary(W)  # (8, P, P)
    # W_eff[(j,p),(k,q)] = sum_i COMB[k,i,j] * Wq[i,p,q]; exactly one i per (j,k)
    return np.einsum("kij,ipq->jpkq", _COMB, Wq).reshape(C, C).astype(np.float32)


def _rope_colperm() -> np.ndarray:
    """colperm[new] = old: within each head, [re0..re63 | im0..im63]."""
    perm = np.zeros(C, dtype=np.int64)
    for h in range(H):
        base = h * HD
        for r in range(HD // 2):
            perm[base + r] = base + 2 * r
            perm[base + HD // 2 + r] = base + 2 * r + 1
    return perm


def prep_inputs(inputs: dict, T: int) -> list[dict]:
    """Build the 8 per-core input maps from the full problem inputs."""
    NT = B * T
    x = np.asarray(inputs["x"], np.float32)[:, :T, :]
    cos = np.asarray(inputs["freqs_cos"], np.float32)[:T]   # [T, 64]
    sin = np.asarray(inputs["freqs_sin"], np.float32)[:T]

    wq_eff = _build_w_eff(np.asarray(inputs["wq"], np.float32))
    wk_eff = _build_w_eff(np.asarray(inputs["wk"], np.float32))
    wv_eff = _build_w_eff(np.asarray(inputs["wv"], np.float32))
    wo_eff = _build_w_eff(np.asarray(inputs["wo"], np.float32))

    perm = _rope_colperm()
    wq_eff = wq_eff[:, perm] * np.float32(HD ** -0.5)
    wk_eff = wk_eff[:, perm]

    # xT [C, NT] -> k-tiles [KT, 128, NT]
    xt = np.ascontiguousarray(
        x.reshape(NT, C).T.reshape(KT, 128, NT).astype(BF16))

    # rope tables, duplicated-half layout [128, T]
    cosd = np.empty((128, T), np.float32)
    cosd[0:64] = cos.T
    cosd[64:128] = cos.T
    sind = np.empty((128, T), np.float32)
    sind[0:64] = -sin.T
    sind[64:128] = sin.T
    cosd = cosd.astype(BF16)
    sind = sind.astype(BF16)

    tri = np.triu(np.ones((128, 128), np.float32)).astype(BF16)  # [s,q] s<=q

    def blocks(w_eff: np.ndarray, c: int) -> np.ndarray:
        blk = w_eff[:, c * P:(c + 1) * P]                  # [C, 256]
        return np.ascontiguousarray(blk.reshape(KT, 128, P).astype(BF16))

    in_maps = []
    for c in range(N_CORES):
        # o-proj row-block for component c: [256, C] -> [2, 128, C]
        wo_rows = np.ascontiguousarray(
            wo_eff[c * P:(c + 1) * P, :].reshape(2, 128, C).astype(BF16))
        in_maps.append({
            "xt": xt,
            "wq": blocks(wq_eff, c),
            "wk": blocks(wk_eff, c),
            "wv": blocks(wv_eff, c),
            "wo": wo_rows,
            "cosd": cosd,
            "sind": sind,
            "tri": tri,
        })
    return in_maps


# ------------------------------------------------------------- device program
def build_nc(T: int = T_FULL, n_cores: int = N_CORES):
    NT = B * T
    ST = T // 128            # s-tiles per batch
    NST = NT // 128
    TCH = min(512, T)        # token chunk; must not cross a batch boundary
    NCH = NT // TCH
    bf16 = mybir.dt.bfloat16
    f32 = mybir.dt.float32

    nc = bacc.Bacc("TRN2", target_bir_lowering=False, debug=False,
                   num_devices=n_cores)

    xt_d = nc.dram_tensor("xt", [KT, 128, NT], bf16, kind="ExternalInput")
    wq_d = nc.dram_tensor("wq", [KT, 128, P], bf16, kind="ExternalInput")
    wk_d = nc.dram_tensor("wk", [KT, 128, P], bf16, kind="ExternalInput")
    wv_d = nc.dram_tensor("wv", [KT, 128, P], bf16, kind="ExternalInput")
    wo_d = nc.dram_tensor("wo", [2, 128, C], bf16, kind="ExternalInput")
    cos_d = nc.dram_tensor("cosd", [128, T], bf16, kind="ExternalInput")
    sin_d = nc.dram_tensor("sind", [128, T], bf16, kind="ExternalInput")
    tri_d = nc.dram_tensor("tri", [128, 128], bf16, kind="ExternalInput")
    out_d = nc.dram_tensor("outt", [C, NT], bf16, kind="ExternalOutput")

    with tile.TileContext(nc) as tc:
        with (
            tc.tile_pool(name="consts", bufs=1) as consts,
            tc.tile_pool(name="persist", bufs=1) as persist,
        ):
            # ================= phase 1: projections + rope =================
            with (
                nc.named_scope("proj"),
                tc.tile_pool(name="xts", bufs=2) as xts_pool,
                tc.tile_pool(name="rope", bufs=3) as rope_pool,
                tc.tile_pool(name="ps1", bufs=3, space="PSUM") as ps1,
                tc.tile_pool(name="psv", bufs=3, space="PSUM") as psv,
            ):
                # ---- resident constants
                wq_s = consts.tile([128, KT, P], bf16, tag="wq")
                wk_s = consts.tile([128, KT, P], bf16, tag="wk")
                wv_s = consts.tile([128, KT, P], bf16, tag="wv")
                wo_s = consts.tile([128, 2, C], bf16, tag="wo")
                # DMA order tuned to the consumption schedule: wq+xt0 in
                # interleaved k-quarters (first matmuls start after ~1/4),
                # then cos/sin (rope frees the proj PSUM slots), wk, the
                # chunk-1 xt prefetch, and only then the later-used weights.
                xt0_s = xts_pool.tile([128, KT, TCH], bf16, tag="xt")
                for kq in range(0, KT, 4):
                    nc.sync.dma_start(
                        out=wq_s[:, kq:kq + 4, :],
                        in_=wq_d.ap()[kq:kq + 4].rearrange("k p n -> p k n"))
                    nc.sync.dma_start(
                        out=xt0_s[:, kq:kq + 4, :],
                        in_=xt_d.ap()[kq:kq + 4, :, 0:TCH]
                        .rearrange("k p n -> p k n"))
                cos_s = consts.tile([128, T], bf16, tag="cos")
                sin_s = consts.tile([128, T], bf16, tag="sin")
                nc.sync.dma_start(out=cos_s, in_=cos_d.ap())
                nc.sync.dma_start(out=sin_s, in_=sin_d.ap())
                nc.sync.dma_start(out=wk_s,
                                  in_=wk_d.ap().rearrange("k p n -> p k n"))
                xt1_s = None
                if NCH > 1:
                    xt1_s = xts_pool.tile([128, KT, TCH], bf16, tag="xt")
                    nc.sync.dma_start(out=xt1_s,
                                      in_=xt_d.ap()[:, :, TCH:2 * TCH]
                                      .rearrange("k p n -> p k n"))
                nc.sync.dma_start(out=wv_s,
                                  in_=wv_d.ap().rearrange("k p n -> p k n"))
                tri_s = consts.tile([128, 128], bf16, tag="tri")
                nc.sync.dma_start(out=tri_s, in_=tri_d.ap())
                nc.sync.dma_start(out=wo_s,
                                  in_=wo_d.ap().rearrange("k p n -> p k n"))
                ident = consts.tile([128, 128], bf16, tag="ident")
                make_identity(nc, ident[:])

                # ---- persistent activations
                qt_s = persist.tile([128, 2, NT], bf16, tag="qt")  # [d, head, tok]
                kt_s = persist.tile([128, 2, NT], bf16, tag="kt")
                v_s = persist.tile([128, NST, 2, 132], bf16, tag="v")
                nc.vector.memset(v_s[:, :, :, 128:129], 1.0)

                for ch in range(NCH):
                    t0 = ch * TCH
                    pos0 = t0 % T          # position within batch
                    if ch == 0:
                        xt_s = xt0_s
                    elif ch == 1:
                        xt_s = xt1_s
                    else:
                        xt_s = xts_pool.tile([128, KT, TCH], bf16, tag="xt")
                        nc.sync.dma_start(out=xt_s,
                                          in_=xt_d.ap()[:, :, t0:t0 + TCH]
                                          .rearrange("k p n -> p k n"))

                    # q/k projections (channel-major out) + rope
                    for w_s, dst in ((wq_s, qt_s), (wk_s, kt_s)):
                        for a in range(2):  # head within component
                            ps_q = ps1.tile([128, TCH], f32, tag="psq")
                            for k in range(KT):
                                nc.tensor.matmul(
                                    ps_q[:],
                                    lhsT=w_s[:, k, a * 128:(a + 1) * 128],
                                    rhs=xt_s[:, k, :],
                                    start=(k == 0), stop=(k == KT - 1))
                            # rope: out = q * cos_dup + swap(q) * sin_signed
                            # (swap of partition halves must go through DMA --
                            # compute engines cannot move data across partitions)
                            q_sb = rope_pool.tile([128, TCH], bf16, tag="qsb")
                            nc.scalar.copy(out=q_sb[:], in_=ps_q[:])
                            qsw = rope_pool.tile([128, TCH], bf16, tag="qsw")
                            nc.sync.dma_start(out=qsw[0:64, :], in_=q_sb[64:128, :])
                            nc.sync.dma_start(out=qsw[64:128, :], in_=q_sb[0:64, :])
                            t1 = rope_pool.tile([128, TCH], bf16, tag="t1")
                            nc.vector.tensor_mul(
                                t1[:], q_sb[:], cos_s[:, pos0:pos0 + TCH])
                            t2 = rope_pool.tile([128, TCH], bf16, tag="t2")
                            nc.vector.tensor_mul(
                                t2[:], qsw[:], sin_s[:, pos0:pos0 + TCH])
                            nc.vector.tensor_add(
                                dst[:, a, t0:t0 + TCH], t1[:], t2[:])

                    # v projection (natural layout)
                    for st in range(TCH // 128):
                        stg = t0 // 128 + st
                        ps_v = psv.tile([128, P], f32, tag="psv")
                        for k in range(KT):
                            nc.tensor.matmul(
                                ps_v[:],
                                lhsT=xt_s[:, k, st * 128:(st + 1) * 128],
                                rhs=wv_s[:, k, :],
                                start=(k == 0), stop=(k == KT - 1))
                        # [t, (head d)] -> v_s[:, stg, head, 0:128]
                        nc.vector.tensor_copy(
                            v_s[:, stg, :, 0:128],
                            ps_v[:].rearrange("p (a d) -> p a d", a=2))

            # ====== phases 2+3: causal attention + partial o-proj, per batch
            # o-proj for batch b is emitted right after batch b's attention,
            # so its matmuls fill attention-phase gaps and its 32MB output
            # DMA spreads over the rest of the kernel instead of the tail.
            ystages = {}
            with (
                tc.tile_pool(name="pt", bufs=ST) as pt_pool,
                tc.tile_pool(name="att_small", bufs=4) as small_pool,
                tc.tile_pool(name="ysb", bufs=ST) as ysb_pool,
                tc.tile_pool(name="ostage", bufs=4) as o_pool,
                tc.tile_pool(name="ps_s", bufs=3, space="PSUM") as ps_s,
                tc.tile_pool(name="ps_y", bufs=3, space="PSUM") as ps_y,
                tc.tile_pool(name="ps_t", bufs=2, space="PSUM") as ps_t,
            ):
                for b in range(B):
                    for a in range(2):
                        qh = qt_s[:, a, b * T:(b + 1) * T]   # [128, T]
                        kh = kt_s[:, a, b * T:(b + 1) * T]
                        # --- A: scores^T + exp per s-tile
                        pts = [None] * ST
                        for j in range(ST):
                            pt_j = pt_pool.tile([128, T], bf16, tag="ptj")
                            pts[j] = pt_j
                            q0 = 128 * j
                            while q0 < T:
                                w = min(512, T - q0)
                                ps = ps_s.tile([128, 512], f32, tag="pss")
                                nc.tensor.matmul(
                                    ps[:, 0:w],
                                    lhsT=kh[:, 128 * j:128 * (j + 1)],
                                    rhs=qh[:, q0:q0 + w],
                                    start=True, stop=True)
                                nc.scalar.activation(
                                    out=pt_j[:, q0:q0 + w], in_=ps[:, 0:w],
                                    func=mybir.ActivationFunctionType.Exp)
                                q0 += w
                            # causal mask on the diagonal 128x128 block
                            nc.vector.tensor_mul(
                                pt_j[:, 128 * j:128 * (j + 1)],
                                pt_j[:, 128 * j:128 * (j + 1)], tri_s[:])
                        # --- B: y = P^T.T @ [v|1], normalize, transpose
                        y_stage = persist.tile([128, T], bf16,
                                               tag=f"ystage{b}{a}")
                        ystages[(b, a)] = y_stage
                        y_sbs = [None] * ST
                        # descending i: long accumulation chains first, so the
                        # PE stays ahead of the DVE normalize chain throughout
                        for i in reversed(range(ST)):
                            psy = ps_y.tile([128, 132], f32, tag="psy")
                            for j in range(i + 1):
                                nc.tensor.matmul(
                                    psy[:, 0:129],
                                    lhsT=pts[j][:, 128 * i:128 * (i + 1)],
                                    rhs=v_s[:, b * ST + j, a, 0:129],
                                    start=(j == 0), stop=(j == i))
                            recip = small_pool.tile([128, 1], f32, tag="recip")
                            nc.vector.reciprocal(recip[:], psy[:, 128:129])
                            y_sb = ysb_pool.tile([128, 128], bf16, tag="ysb")
                            nc.vector.tensor_scalar_mul(
                                y_sb[:], psy[:, 0:128], recip[:])
                            y_sbs[i] = y_sb
                        # transposes batched after the PV chain so the PE
                        # never stalls waiting on the DVE normalize
                        for i in range(ST):
                            pst = ps_t.tile([128, 128], bf16, tag="pst")
                            nc.tensor.transpose(pst[:], y_sbs[i][:], ident[:])
                            # NB: must stay on DVE -- ACT reading bf16 PSUM
                            # hard-faulted the exec unit on HW
                            nc.vector.tensor_copy(
                                y_stage[:, 128 * i:128 * (i + 1)], pst[:])

                    # --- partial o-proj for this batch:
                    # outT_partial[cout, t] = sum_{cin in c} Wo[cin,cout] yT[cin,t]
                    for lch in range(T // TCH):
                        lt0 = lch * TCH
                        t0 = b * T + lt0
                        for m in range(C // 128):        # 16 cout tiles
                            ps = ps_s.tile([128, 512], f32, tag="pss")
                            for k in range(2):           # cin k-tiles (= heads)
                                nc.tensor.matmul(
                                    ps[:, 0:TCH],
                                    lhsT=wo_s[:, k, m * 128:(m + 1) * 128],
                                    rhs=ystages[(b, k)][:, lt0:lt0 + TCH],
                                    start=(k == 0), stop=(k == 1))
                            # copies split DVE/ACT: ACT's next exp stream is
                            # sequenced behind these matmuls anyway, so it is
                            # idle here, and DVE alone (~6.2us/chunk) sits
                            # right at the PE rate (~6.8us/chunk)
                            o_sb = o_pool.tile([128, TCH], bf16, tag="osb")
                            if m % 2 == 0:
                                nc.vector.tensor_copy(o_sb[:], ps[:, 0:TCH])
                            else:
                                nc.scalar.copy(out=o_sb[:], in_=ps[:, 0:TCH])
                            nc.sync.dma_start(
                                out=out_d.ap()[m * 128:(m + 1) * 128,
                                               t0:t0 + TCH],
                                in_=o_sb[:])

    nc.compile()
    return nc


# ------------------------------------------------------------------ entrypoint
_NC_CACHE: dict = {}


def _get_nc(T: int):
    if T not in _NC_CACHE:
        _NC_CACHE[T] = build_nc(T)
    return _NC_CACHE[T]


def assemble_output(results: list[dict], T: int = T_FULL) -> np.ndarray:
    # unshard = sum of the 8 tensor-parallel partial projections (bf16 -> f32)
    outT = results[0]["outt"].astype(np.float32)                # [C, NT]
    for r in results[1:]:
        outT += r["outt"].astype(np.float32)
    return np.ascontiguousarray(outT.T).reshape(B, T, C).astype(np.float32)


def kernel(**inputs) -> np.ndarray:
    nc = _get_nc(T_FULL)
    in_maps = prep_inputs(inputs, T_FULL)
    res = run_bass_kernel_spmd(nc, in_maps, list(range(N_CORES)))
    return assemble_output(res.results, T_FULL)



# revision 2
# speedup vs baseline: 1.1004x; 1.1004x over previous
"""Trainium2 Bass kernel for the octonion causal self-attention block.

Strategy (8 NeuronCores, SPMD):
  Each core owns one octonion component c (= heads 2c, 2c+1).
  - Host prep: ternary-quantize weights exactly as the reference does and
    keep them as EXACT {-1,0,+1} matrices (fp8/bf16 representable); the
    scalar scales are folded into (a) the exp() activation scale for
    s_q*s_k/sqrt(HD), and (b) the host-side unshard for s_v*s_o.
    Assemble the effective [C, C] block matrices (sign/permutation
    combine folded in), permute q/k output channels into RoPE
    split-layout, and transpose x to channel-major xT in chunk-
    contiguous layout (one fp8 copy for q/k, one bf16 copy for v).
  - Device phase 0: a short burst of warm-up matmuls on a zero tile so
    the PE HAM clock-gate reaches K=8/8 during the initial DMA wait.
  - Device phase 1: q/k projections in fp8 DoubleRow (2x contraction per
    cycle; ternary weights are exact in fp8, only x carries quantization
    noise which the softmax largely cancels), v projection in bf16.
    RoPE applied on the fly; qT/kT kept channel-major in SBUF.
  - Device phase 2: causal attention per (batch, head): S^T = K^T Q per
    s-tile, exp with the folded score scale (no max subtraction --
    scores are O(1) bounded), triangular mask on the diagonal tile,
    P^T V with an appended ones-column in V giving the softmax
    denominator for free, per-partition normalize, PE-transpose back to
    channel-major yT (kept in SBUF).
  - Device phase 3: PARTIAL output projection with ternary bf16 Wo:
    out_partial^T = Wo[rows of component c]^T @ yT_c -- no collective.
    The host scales by s_v*s_o and sums the 8 partial outputs while
    unsharding.
"""

import numpy as np
import ml_dtypes

import concourse.bass as bass
import concourse.tile as tile
from concourse import bacc, mybir
from concourse.bass_utils import run_bass_kernel_spmd
from concourse.masks import make_identity

# ---------------------------------------------------------------- problem dims
B, T_FULL, C, H = 2, 2048, 2048, 16
HD = C // H          # 128
P = C // 8           # 256
N_CORES = 8
KT = C // 128        # 16 contraction k-tiles

OCT_SIGN = np.array([
    [1, 1, 1, 1, 1, 1, 1, 1],
    [1,-1, 1,-1, 1,-1,-1, 1],
    [1,-1,-1, 1, 1, 1,-1,-1],
    [1, 1,-1,-1, 1,-1, 1,-1],
    [1,-1,-1,-1,-1, 1, 1, 1],
    [1, 1,-1, 1,-1,-1,-1, 1],
    [1, 1, 1,-1,-1, 1,-1,-1],
    [1,-1, 1, 1,-1,-1, 1,-1]], dtype=np.float32)
OCT_IDX = np.array([
    [0,1,2,3,4,5,6,7],
    [1,0,3,2,5,4,7,6],
    [2,3,0,1,6,7,4,5],
    [3,2,1,0,7,6,5,4],
    [4,5,6,7,0,1,2,3],
    [5,4,7,6,1,0,3,2],
    [6,7,4,5,2,3,0,1],
    [7,6,5,4,3,2,1,0]], dtype=np.int32)
_COMB = np.zeros((8, 8, 8), np.float32)
for _i in range(8):
    for _j in range(8):
        _COMB[OCT_IDX[_i, _j], _i, _j] = OCT_SIGN[_i, _j]

BF16 = ml_dtypes.bfloat16
FP8 = ml_dtypes.float8_e4m3


# ------------------------------------------------------------------- host prep
def _ternary(W: np.ndarray) -> tuple[np.ndarray, np.float32]:
    """Reference's ternary quantization, returned as exact {-1,0,1} + scale."""
    try:
        import jax
        import jax.numpy as jnp
        cpu = jax.local_devices(backend="cpu")[0]
        with jax.default_device(cpu):
            Wj = jnp.asarray(W)
            s = jnp.mean(jnp.abs(Wj)) + 1e-8
            t = jnp.round(jnp.clip(Wj / s, -1.0, 1.0))
            return np.asarray(t, np.float32), np.float32(s)
    except Exception:
        s = np.float32(np.mean(np.abs(W.astype(np.float32)))) + np.float32(1e-8)
        return np.rint(np.clip(W / s, -1.0, 1.0)).astype(np.float32), s


def _build_t_eff(W: np.ndarray) -> tuple[np.ndarray, np.float32]:
    """[8,P,P] weights -> exact ternary effective [C, C] + scale."""
    Tq, s = _ternary(W)  # (8, P, P) in {-1,0,1}
    # T_eff[(j,p),(k,q)] = sum_i COMB[k,i,j] * Tq[i,p,q]; exactly one i per (j,k)
    return (np.einsum("kij,ipq->jpkq", _COMB, Tq).reshape(C, C), s)


def _rope_colperm() -> np.ndarray:
    """colperm[new] = old: within each head, [re0..re63 | im0..im63]."""
    perm = np.zeros(C, dtype=np.int64)
    for h in range(H):
        base = h * HD
        for r in range(HD // 2):
            perm[base + r] = base + 2 * r
            perm[base + HD // 2 + r] = base + 2 * r + 1
    return perm


def prep_inputs(inputs: dict, T: int) -> list[dict]:
    """Build the 8 per-core input maps from the full problem inputs."""
    NT = B * T
    TCH = min(512, T)
    NCH = NT // TCH
    x = np.asarray(inputs["x"], np.float32)[:, :T, :]
    cos = np.asarray(inputs["freqs_cos"], np.float32)[:T]   # [T, 64]
    sin = np.asarray(inputs["freqs_sin"], np.float32)[:T]

    tq_eff, s_q = _build_t_eff(np.asarray(inputs["wq"], np.float32))
    tk_eff, s_k = _build_t_eff(np.asarray(inputs["wk"], np.float32))
    tv_eff, s_v = _build_t_eff(np.asarray(inputs["wv"], np.float32))
    to_eff, s_o = _build_t_eff(np.asarray(inputs["wo"], np.float32))

    perm = _rope_colperm()
    tq_eff = tq_eff[:, perm]
    tk_eff = tk_eff[:, perm]

    # xT [C, NT] in chunk-contiguous layout [NCH, 128, KT, TCH]:
    # element [ch, p, k, t] = xT[k*128+p, ch*TCH+t] -> per-partition lines
    # are KT*TCH contiguous elements (one DMA descriptor per partition).
    xt = x.reshape(NT, C).T.reshape(KT, 128, NCH, TCH).transpose(2, 1, 0, 3)
    xt8 = np.ascontiguousarray(xt.astype(FP8))
    xtb = np.ascontiguousarray(xt.astype(BF16))

    # rope tables, duplicated-half layout [128, T]
    cosd = np.empty((128, T), np.float32)
    cosd[0:64] = cos.T
    cosd[64:128] = cos.T
    sind = np.empty((128, T), np.float32)
    sind[0:64] = -sin.T
    sind[64:128] = sin.T
    cosd = cosd.astype(BF16)
    sind = sind.astype(BF16)

    tri = np.triu(np.ones((128, 128), np.float32)).astype(BF16)  # [s,q] s<=q

    # folded score scale for the exp() activation
    alpha = np.full((128, 1), s_q * s_k * (HD ** -0.5), np.float32)
    out_scale = float(s_v) * float(s_o)

    def wblocks(t_eff: np.ndarray, c: int, dt) -> np.ndarray:
        # [C, 256] block -> [128, KT, 256] (partition-major, contiguous lines)
        blk = t_eff[:, c * P:(c + 1) * P].reshape(KT, 128, P).transpose(1, 0, 2)
        return np.ascontiguousarray(blk.astype(dt))

    in_maps = []
    for c in range(N_CORES):
        # o-proj row-block for component c: [256, C] -> [2, 128, C]
        wo_rows = np.ascontiguousarray(
            to_eff[c * P:(c + 1) * P, :].reshape(2, 128, C).astype(BF16))
        in_maps.append({
            "xt8": xt8,
            "xtb": xtb,
            "wq": wblocks(tq_eff, c, FP8),
            "wk": wblocks(tk_eff, c, FP8),
            "wv": wblocks(tv_eff, c, BF16),
            "wo": wo_rows,
            "cosd": cosd,
            "sind": sind,
            "tri": tri,
            "alpha": alpha,
        })
    return in_maps, out_scale


# ------------------------------------------------------------- device program
def build_nc(T: int = T_FULL, n_cores: int = N_CORES):
    NT = B * T
    ST = T // 128            # s-tiles per batch
    NST = NT // 128
    TCH = min(512, T)        # token chunk; must not cross a batch boundary
    NCH = NT // TCH
    bf16 = mybir.dt.bfloat16
    fp8 = mybir.dt.float8e4
    f32 = mybir.dt.float32
    DR = mybir.MatmulPerfMode.DoubleRow

    nc = bacc.Bacc("TRN2", target_bir_lowering=False, debug=False,
                   num_devices=n_cores)

    xt8_d = nc.dram_tensor("xt8", [NCH, 128, KT, TCH], fp8, kind="ExternalInput")
    xtb_d = nc.dram_tensor("xtb", [NCH, 128, KT, TCH], bf16, kind="ExternalInput")
    wq_d = nc.dram_tensor("wq", [128, KT, P], fp8, kind="ExternalInput")
    wk_d = nc.dram_tensor("wk", [128, KT, P], fp8, kind="ExternalInput")
    wv_d = nc.dram_tensor("wv", [128, KT, P], bf16, kind="ExternalInput")
    wo_d = nc.dram_tensor("wo", [2, 128, C], bf16, kind="ExternalInput")
    cos_d = nc.dram_tensor("cosd", [128, T], bf16, kind="ExternalInput")
    sin_d = nc.dram_tensor("sind", [128, T], bf16, kind="ExternalInput")
    tri_d = nc.dram_tensor("tri", [128, 128], bf16, kind="ExternalInput")
    alpha_d = nc.dram_tensor("alpha", [128, 1], f32, kind="ExternalInput")
    out_d = nc.dram_tensor("outt", [C, NT], bf16, kind="ExternalOutput")

    with tile.TileContext(nc) as tc:
        with (
            tc.tile_pool(name="consts", bufs=1) as consts,
            tc.tile_pool(name="persist", bufs=1) as persist,
        ):
            # ================= phase 1: projections + rope =================
            with (
                nc.named_scope("proj"),
                tc.tile_pool(name="xt8s", bufs=2) as xt8_pool,
                tc.tile_pool(name="xtbs", bufs=2) as xtb_pool,
                tc.tile_pool(name="rope", bufs=3) as rope_pool,
                tc.tile_pool(name="ps1", bufs=3, space="PSUM") as ps1,
                tc.tile_pool(name="psv", bufs=3, space="PSUM") as psv,
            ):
                # ---- PE warm-up: ~3.5us of junk matmuls with no DMA deps so
                # the HAM clock-gate flips to K=8/8 while inputs stream in.
                warm = consts.tile([128, 512], bf16, tag="warm")
                nc.vector.memset(warm[:], 0.0)
                ps_w = ps1.tile([128, 512], f32, tag="psq")
                for _ in range(8):
                    nc.tensor.matmul(ps_w[:], lhsT=warm[:, 0:128], rhs=warm[:],
                                     start=True, stop=True)

                # ---- resident constants (DMA order = consumption order)
                alpha_s = consts.tile([128, 1], f32, tag="alpha")
                nc.sync.dma_start(out=alpha_s, in_=alpha_d.ap())
                wq_s = consts.tile([128, KT, P], fp8, tag="wq")
                wk_s = consts.tile([128, KT, P], fp8, tag="wk")
                wv_s = consts.tile([128, KT, P], bf16, tag="wv")
                wo_s = consts.tile([128, 2, C], bf16, tag="wo")
                xt8_0 = xt8_pool.tile([128, KT, TCH], fp8, tag="xt8")
                for kq in range(0, KT, 4):
                    nc.sync.dma_start(out=wq_s[:, kq:kq + 4, :],
                                      in_=wq_d.ap()[:, kq:kq + 4, :])
                    nc.sync.dma_start(out=xt8_0[:, kq:kq + 4, :],
                                      in_=xt8_d.ap()[0, :, kq:kq + 4, :])
                cos_s = consts.tile([128, T], bf16, tag="cos")
                sin_s = consts.tile([128, T], bf16, tag="sin")
                nc.sync.dma_start(out=cos_s, in_=cos_d.ap())
                nc.sync.dma_start(out=sin_s, in_=sin_d.ap())
                nc.sync.dma_start(out=wk_s, in_=wk_d.ap())
                xtb_0 = xtb_pool.tile([128, KT, TCH], bf16, tag="xtb")
                nc.sync.dma_start(out=xtb_0, in_=xtb_d.ap()[0])
                nc.sync.dma_start(out=wv_s, in_=wv_d.ap())
                xt8_1 = xt8_pool.tile([128, KT, TCH], fp8, tag="xt8")
                nc.sync.dma_start(out=xt8_1, in_=xt8_d.ap()[1])
                xtb_1 = xtb_pool.tile([128, KT, TCH], bf16, tag="xtb")
                nc.sync.dma_start(out=xtb_1, in_=xtb_d.ap()[1])
                tri_s = consts.tile([128, 128], bf16, tag="tri")
                nc.sync.dma_start(out=tri_s, in_=tri_d.ap())
                nc.sync.dma_start(out=wo_s,
                                  in_=wo_d.ap().rearrange("k p n -> p k n"))
                ident = consts.tile([128, 128], bf16, tag="ident")
                make_identity(nc, ident[:])

                # ---- persistent activations
                qt_s = persist.tile([128, 2, NT], bf16, tag="qt")  # [d, head, tok]
                kt_s = persist.tile([128, 2, NT], bf16, tag="kt")
                v_s = persist.tile([128, NST, 2, 132], bf16, tag="v")
                nc.vector.memset(v_s[:, :, :, 128:129], 1.0)

                for ch in range(NCH):
                    t0 = ch * TCH
                    pos0 = t0 % T          # position within batch
                    if ch == 0:
                        xt8_s, xtb_s = xt8_0, xtb_0
                    elif ch == 1:
                        xt8_s, xtb_s = xt8_1, xtb_1
                    else:
                        xt8_s = xt8_pool.tile([128, KT, TCH], fp8, tag="xt8")
                        nc.sync.dma_start(out=xt8_s, in_=xt8_d.ap()[ch])
                        xtb_s = xtb_pool.tile([128, KT, TCH], bf16, tag="xtb")
                        nc.sync.dma_start(out=xtb_s, in_=xtb_d.ap()[ch])

                    # q/k projections: fp8 DoubleRow (channel-major out) + rope
                    for w_s, dst in ((wq_s, qt_s), (wk_s, kt_s)):
                        for a in range(2):  # head within component
                            ps_q = ps1.tile([128, TCH], f32, tag="psq")
                            for kp in range(KT // 2):
                                nc.tensor.matmul(
                                    ps_q[:],
                                    lhsT=w_s[:, 2 * kp:2 * kp + 2,
                                             a * 128:(a + 1) * 128],
                                    rhs=xt8_s[:, 2 * kp:2 * kp + 2, :],
                                    start=(kp == 0), stop=(kp == KT // 2 - 1),
                                    perf_mode=DR)
                            # rope: out = q * cos_dup + swap(q) * sin_signed
                            # (swap of partition halves must go through DMA --
                            # compute engines cannot move data across partitions)
                            q_sb = rope_pool.tile([128, TCH], bf16, tag="qsb")
                            nc.scalar.copy(out=q_sb[:], in_=ps_q[:])
                            qsw = rope_pool.tile([128, TCH], bf16, tag="qsw")
                            nc.sync.dma_start(out=qsw[0:64, :], in_=q_sb[64:128, :])
                            nc.sync.dma_start(out=qsw[64:128, :], in_=q_sb[0:64, :])
                            t1 = rope_pool.tile([128, TCH], bf16, tag="t1")
                            nc.vector.tensor_mul(
                                t1[:], q_sb[:], cos_s[:, pos0:pos0 + TCH])
                            t2 = rope_pool.tile([128, TCH], bf16, tag="t2")
                            nc.vector.tensor_mul(
                                t2[:], qsw[:], sin_s[:, pos0:pos0 + TCH])
                            nc.vector.tensor_add(
                                dst[:, a, t0:t0 + TCH], t1[:], t2[:])

                    # v projection in bf16 (natural layout); accuracy-critical,
                    # so no fp8 on this path
                    for st in range(TCH // 128):
                        stg = t0 // 128 + st
                        ps_v = psv.tile([128, P], f32, tag="psv")
                        for k in range(KT):
                            nc.tensor.matmul(
                                ps_v[:],
                                lhsT=xtb_s[:, k, st * 128:(st + 1) * 128],
                                rhs=wv_s[:, k, :],
                                start=(k == 0), stop=(k == KT - 1))
                        # [t, (head d)] -> v_s[:, stg, head, 0:128]
                        nc.vector.tensor_copy(
                            v_s[:, stg, :, 0:128],
                            ps_v[:].rearrange("p (a d) -> p a d", a=2))

            # ====== phases 2+3: causal attention + partial o-proj, per batch
            # o-proj for batch b is emitted right after batch b's attention,
            # so its matmuls fill attention-phase gaps and its 32MB output
            # DMA spreads over the rest of the kernel instead of the tail.
            ystages = {}
            with (
                tc.tile_pool(name="pt", bufs=1) as pt_pool,
                tc.tile_pool(name="att_small", bufs=4) as small_pool,
                tc.tile_pool(name="ysb", bufs=ST) as ysb_pool,
                tc.tile_pool(name="ostage", bufs=4) as o_pool,
                tc.tile_pool(name="ps_s", bufs=3, space="PSUM") as ps_s,
                tc.tile_pool(name="ps_y", bufs=3, space="PSUM") as ps_y,
                tc.tile_pool(name="ps_t", bufs=2, space="PSUM") as ps_t,
            ):
                for b in range(B):
                    for a in range(2):
                        qh = qt_s[:, a, b * T:(b + 1) * T]   # [128, T]
                        kh = kt_s[:, a, b * T:(b + 1) * T]
                        # --- A: scores^T + exp per s-tile (triangular tiles:
                        # pt_j holds columns q in [128j, T))
                        pts = [None] * ST
                        for j in range(ST):
                            wj = T - 128 * j
                            pt_j = pt_pool.tile([128, wj], bf16, tag=f"pt{j}")
                            pts[j] = pt_j
                            q0 = 128 * j
                            while q0 < T:
                                w = min(512, T - q0)
                                ps = ps_s.tile([128, 512], f32, tag="pss")
                                nc.tensor.matmul(
                                    ps[:, 0:w],
                                    lhsT=kh[:, 128 * j:128 * (j + 1)],
                                    rhs=qh[:, q0:q0 + w],
                                    start=True, stop=True)
                                # exp with the folded s_q*s_k/sqrt(HD) scale
                                nc.scalar.activation(
                                    out=pt_j[:, q0 - 128 * j:q0 - 128 * j + w],
                                    in_=ps[:, 0:w],
                                    func=mybir.ActivationFunctionType.Exp,
                                    scale=alpha_s[:, 0:1])
                                q0 += w
                            # causal mask on the diagonal 128x128 block
                            nc.vector.tensor_mul(
                                pt_j[:, 0:128], pt_j[:, 0:128], tri_s[:])
                        # --- B: y = P^T.T @ [v|1], normalize, transpose
                        y_stage = persist.tile([128, T], bf16,
                                               tag=f"ystage{b}{a}")
                        ystages[(b, a)] = y_stage
                        y_sbs = [None] * ST
                        # descending i: long accumulation chains first, so the
                        # PE stays ahead of the DVE normalize chain throughout
                        for i in reversed(range(ST)):
                            psy = ps_y.tile([128, 132], f32, tag="psy")
                            for j in range(i + 1):
                                nc.tensor.matmul(
                                    psy[:, 0:129],
                                    lhsT=pts[j][:, 128 * (i - j):
                                                128 * (i - j) + 128],
                                    rhs=v_s[:, b * ST + j, a, 0:129],
                                    start=(j == 0), stop=(j == i))
                            recip = small_pool.tile([128, 1], f32, tag="recip")
                            nc.vector.reciprocal(recip[:], psy[:, 128:129])
                            y_sb = ysb_pool.tile([128, 128], bf16, tag="ysb")
                            nc.vector.tensor_scalar_mul(
                                y_sb[:], psy[:, 0:128], recip[:])
                            y_sbs[i] = y_sb
                        # transposes batched after the PV chain so the PE
                        # never stalls waiting on the DVE normalize
                        for i in range(ST):
                            pst = ps_t.tile([128, 128], bf16, tag="pst")
                            nc.tensor.transpose(pst[:], y_sbs[i][:], ident[:])
                            # NB: must stay on DVE -- ACT reading bf16 PSUM
                            # hard-faulted the exec unit on HW
                            nc.vector.tensor_copy(
                                y_stage[:, 128 * i:128 * (i + 1)], pst[:])

                    # --- partial o-proj for this batch:
                    # outT_partial[cout, t] = sum_{cin in c} Wo[cin,cout] yT[cin,t]
                    for lch in range(T // TCH):
                        lt0 = lch * TCH
                        t0 = b * T + lt0
                        for m in range(C // 128):        # 16 cout tiles
                            ps = ps_s.tile([128, 512], f32, tag="pss")
                            for k in range(2):           # cin k-tiles (= heads)
                                nc.tensor.matmul(
                                    ps[:, 0:TCH],
                                    lhsT=wo_s[:, k, m * 128:(m + 1) * 128],
                                    rhs=ystages[(b, k)][:, lt0:lt0 + TCH],
                                    start=(k == 0), stop=(k == 1))
                            # copies split DVE/ACT: ACT's next exp stream is
                            # sequenced behind these matmuls anyway, so it is
                            # idle here, and DVE alone (~6.2us/chunk) sits
                            # right at the PE rate (~6.8us/chunk)
                            o_sb = o_pool.tile([128, TCH], bf16, tag="osb")
                            if m % 2 == 0:
                                nc.vector.tensor_copy(o_sb[:], ps[:, 0:TCH])
                            else:
                                nc.scalar.copy(out=o_sb[:], in_=ps[:, 0:TCH])
                            nc.sync.dma_start(
                                out=out_d.ap()[m * 128:(m + 1) * 128,
                                               t0:t0 + TCH],
                                in_=o_sb[:])

    nc.compile()
    return nc


# ------------------------------------------------------------------ entrypoint
_NC_CACHE: dict = {}


def _get_nc(T: int):
    if T not in _NC_CACHE:
        _NC_CACHE[T] = build_nc(T)
    return _NC_CACHE[T]


def assemble_output(results: list[dict], T: int = T_FULL,
                    out_scale: float = 1.0) -> np.ndarray:
    # unshard = sum of the 8 tensor-parallel partial projections (bf16 -> f32)
    outT = results[0]["outt"].astype(np.float32)                # [C, NT]
    for r in results[1:]:
        outT += r["outt"].astype(np.float32)
    outT *= np.float32(out_scale)
    return np.ascontiguousarray(outT.T).reshape(B, T, C).astype(np.float32)


def kernel(**inputs) -> np.ndarray:
    nc = _get_nc(T_FULL)
    in_maps, out_scale = prep_inputs(inputs, T_FULL)
    res = run_bass_kernel_spmd(nc, in_maps, list(range(N_CORES)))
    return assemble_output(res.results, T_FULL, out_scale)
